# revision 23
# baseline (speedup 1.0000x reference)
"""Multi-head causal attention (B=4, L=2048, D=1024, H=16, dh=64) on 8 TRN2 NeuronCores.

Sharding: core i handles batch b = i//2 and head-group g = i%2 (8 heads each).
No cross-core communication needed: each core computes o[b, :, g*512:(g+1)*512].

Per-core dataflow (all layouts chosen so matmul contraction is on partitions):
  inputs (host-prepared, bf16, tiled so every DMA is a single contiguous ~1MB read):
    qTt/kTt/vTt [512, 4096]: row lb*128+p, col d*512+c  holds  x[b][lb*512+c, d*128+p]
    wq/wk/wv    [128, 4096]: row p,       col d*512+c  holds  W[d*128+p, c]
  projections (bf16 matmuls, fp32 psum):
    qwT/kwT [128(2 heads x 64dh), L] bf16;  vw_aug [128(Lk sub), 8*65] bf16 with a
    v_mask column appended per head (gives sum-of-exp for free in the PV matmul).
  attention, per q-tile tau of 512: a single software-pipelined stream over
  (hp, b, half) so the PE never waits on the exp chain:
    S^T[k,q] = kwT.T @ qwT per 128-k block (two K=64 heads row-packed in the PE),
    P^T = exp(S^T/8) via ScalarE (psum->sbuf, bf16), causal zeroing only of the
    128x128 diagonal squares via a DVE multiply with one precomputed triangular
    mask tile, then oT[65, 512] += vw_aug.T @ P^T accumulated over k blocks
    (row 64 = sum of exp).  The PV for stream element i is emitted after the S
    for element i+lead, so exp latency is hidden behind the next S matmuls.
    oT is transposed back via PE transpose; rows are scaled by 1/sumexp on DVE.
v_mask is pre-applied to v on host (and to the ones column via vmask_t on device);
q_mask is applied to the returned output on host.  Masks are {0,1} so this is exact.
"""
import numpy as np
import ml_dtypes
from contextlib import ExitStack
from itertools import chain

import concourse.bass as bass
import concourse.tile as tile
from concourse import bacc, mybir
from concourse.bass_utils import run_bass_kernel_spmd
from concourse.masks import make_identity

F32 = mybir.dt.float32
BF16 = mybir.dt.bfloat16
BF16_NP = ml_dtypes.bfloat16

L = 2048          # sequence length
D = 1024          # d_model
COLS = 512        # projection columns per core (8 heads x 64)
NKSUB = L // 128  # 16 k-subtiles
NTAU = L // 512   # 4 q-tiles
NHP = 4           # head pairs per core


def _build_kernel(interleave="drip", sps_bufs=2, pt_bufs=6, reps=1,
                  do_proj=True, do_attn=True, do_mm=True,
                  pj_share=False, unified_psum=False, lead=2, bf16_post=False,
                  pool_copies=False, act_q="sync", spill=True, post_delay=0):
    nc = bacc.Bacc("TRN2", target_bir_lowering=False, debug=False, num_devices=8)

    qTt = nc.dram_tensor("qTt", [512, 4096], BF16, kind="ExternalInput").ap()
    kTt = nc.dram_tensor("kTt", [512, 4096], BF16, kind="ExternalInput").ap()
    vTt = nc.dram_tensor("vTt", [512, 4096], BF16, kind="ExternalInput").ap()
    wq = nc.dram_tensor("wq", [128, 4096], BF16, kind="ExternalInput").ap()
    wk = nc.dram_tensor("wk", [128, 4096], BF16, kind="ExternalInput").ap()
    wv = nc.dram_tensor("wv", [128, 4096], BF16, kind="ExternalInput").ap()
    vmask_t = nc.dram_tensor("vmask_t", [128, NKSUB], F32, kind="ExternalInput").ap()
    out = nc.dram_tensor("out", [L, COLS], F32, kind="ExternalOutput").ap()

    POST = BF16 if bf16_post else F32

    with tile.TileContext(nc) as tc, ExitStack() as ctx:
        sb = ctx.enter_context(tc.tile_pool(name="sb", bufs=1))
        ps = ctx.enter_context(tc.tile_pool(name="ps", bufs=1, space="PSUM"))

        # ---- persistent SBUF tensors ----
        w_t = {t: sb.tile([128, 4096], BF16, tag="w", bufs=3, name=f"w{t}")
               for t in ("q", "k", "v")}
        w_loaded = set()

        def load_weights(tname, split=False):
            if tname in w_loaded:
                return
            w_loaded.add(tname)
            src = {"q": wq, "k": wk, "v": wv}[tname]
            if split:
                # fine-grained first chunk so the d=0 matmuls start early
                nc.sync.dma_start(w_t[tname][:, 0:512], src[:, 0:512])
                nc.sync.dma_start(w_t[tname][:, 512:2048], src[:, 512:2048])
                nc.sync.dma_start(w_t[tname][:, 2048:4096], src[:, 2048:4096])
            else:
                nc.sync.dma_start(w_t[tname][:, 0:2048], src[:, 0:2048])
                nc.sync.dma_start(w_t[tname][:, 2048:4096], src[:, 2048:4096])

        vmask_sb = sb.tile([128, NKSUB], F32, tag="vm")
        nc.sync.dma_start(vmask_sb[:], vmask_t[:])
        ident = sb.tile([128, 128], F32, tag="id")
        make_identity(nc, ident[:])

        # one triangular mask for the 128x128 diagonal squares:
        # mask_sq[p, q] = 1 if q >= p else 0  (keep k <= q)
        mask_sq = sb.tile([128, 128], BF16, tag="mask")
        nc.gpsimd.memset(mask_sq[:], 1.0)
        nc.gpsimd.affine_select(
            out=mask_sq[:], in_=mask_sq[:], compare_op=mybir.AluOpType.is_ge,
            fill=0.0, base=0, channel_multiplier=-1, pattern=[[1, 128]])

        qwT = [sb.tile([128, L], BF16, tag="qwT", bufs=NHP, name=f"qwT{hp}") for hp in range(NHP)]
        kwT = [sb.tile([128, L], BF16, tag="kwT", bufs=NHP, name=f"kwT{hp}") for hp in range(NHP)]
        vw_aug = [sb.tile([128, 8 * 65], BF16, tag="vwa", bufs=NKSUB, name=f"vwa{u}")
                  for u in range(NKSUB)]

        cpeng = nc.gpsimd if pool_copies else nc.vector
        act_tiles = {}

        def proj_setup(tname, lb):
            """Emit the input DMAs for one L-block (idempotent per block)."""
            key = (tname, lb)
            if key in act_tiles:
                return act_tiles[key]
            first = tname == "k" and lb == 0
            load_weights(tname, split=first)
            src = {"q": qTt, "k": kTt, "v": vTt}[tname]
            act = sb.tile([128, 4096], BF16, tag="act", bufs=6, name=f"a{tname}{lb}")
            # activations go on the Activation HWDGE queue so they don't
            # serialize behind weights/stores on the SP queue
            dma_eng = nc.scalar if act_q == "scalar" else nc.sync
            if first:
                # split the very first load so d=0 matmuls start after 128KB
                dma_eng.dma_start(act[:, 0:512],
                                    src[lb * 128:(lb + 1) * 128, 0:512])
                dma_eng.dma_start(act[:, 512:2048],
                                    src[lb * 128:(lb + 1) * 128, 512:2048])
                dma_eng.dma_start(act[:, 2048:4096],
                                    src[lb * 128:(lb + 1) * 128, 2048:4096])
            else:
                dma_eng.dma_start(act[:], src[lb * 128:(lb + 1) * 128, :])
            act_tiles[key] = act
            return act

        def proj_gen(tname, lb, groups=None):
            """Generator of single-matmul projection units for (part of) one
            L-block.  groups selects head-pairs (q/k) or l-subtiles (v) so a
            block can be split across attention streams with per-piece
            deadlines.  Each next() emits one 512-col matmul (plus the
            psum->SBUF copy after a group's last matmul)."""
            act = proj_setup(tname, lb)
            if not do_mm:
                return iter(())
            if groups is None:
                groups = range(NHP)

            def units():
                wt = w_t[tname]
                pj_tag, pj_shape = (("sps", [128, 1024]) if pj_share
                                    else ("pj", [128, 512]))
                if tname != "v":
                    dst = qwT if tname == "q" else kwT
                    for hp in groups:
                        p = ps.tile(pj_shape, F32, tag=pj_tag,
                                    bufs=sps_bufs if pj_share else 2,
                                    name=f"pj{tname}{lb}{hp}")
                        p = p[:, 0:512]
                        for d in range(8):
                            nc.tensor.matmul(p[:],
                                             wt[:, d * 512 + hp * 128:d * 512 + (hp + 1) * 128],
                                             act[:, d * 512:(d + 1) * 512],
                                             start=(d == 0), stop=(d == 7),
                                             skip_group_check=True)
                            if d == 7:
                                cpeng.tensor_copy(
                                    dst[hp][:, lb * 512:(lb + 1) * 512], p[:])
                            yield
                else:
                    for ls in groups:
                        u = lb * 4 + ls
                        p = ps.tile(pj_shape, F32, tag=pj_tag,
                                    bufs=sps_bufs if pj_share else 2,
                                    name=f"pjv{u}")
                        p = p[:, 0:512]
                        for d in range(8):
                            nc.tensor.matmul(p[:],
                                             act[:, d * 512 + ls * 128:d * 512 + ls * 128 + 128],
                                             wt[:, d * 512:(d + 1) * 512],
                                             start=(d == 0), stop=(d == 7),
                                             skip_group_check=True)
                            if d == 7:
                                v3d = vw_aug[u][:].rearrange("p (h c) -> p h c", h=8)
                                cpeng.tensor_copy(
                                    v3d[:, :, 0:64],
                                    p[:].rearrange("p (h c) -> p h c", h=8))
                                cpeng.tensor_copy(
                                    v3d[:, :, 64:65].squeeze(2),
                                    vmask_sb[:, u:u + 1].broadcast_to([128, 8]))
                            yield
            return units()

        def proj_block(tname, lb):
            """Eagerly-drained projection block (preamble / non-drip modes)."""
            for _ in proj_gen(tname, lb):
                pass

        oo_tiles = {}
        otp_tiles = {}

        def emit_S(tau, hp, b, half):
            """S^T matmuls + exp (+ diag-square mask) for one (b, half).
            Returns the pt tile + metadata needed by emit_PV."""
            diag = b >= 2 * tau
            col0 = [128 * max(0, 2 * b + j - 4 * tau) for j in range(2)]
            s = ps.tile([128, 1024], F32, tag="sps", bufs=sps_bufs,
                        name=f"ss{tau}{hp}{b}{half}")
            for j in range(2):
                u = 2 * b + j
                nc.tensor.matmul(
                    s[:, j * 512 + col0[j]:(j + 1) * 512],
                    kwT[hp][64 * half:64 * half + 64, u * 128:(u + 1) * 128],
                    qwT[hp][64 * half:64 * half + 64,
                            tau * 512 + col0[j]:(tau + 1) * 512],
                    start=True, stop=True, skip_group_check=True,
                    tile_position=(64 * half, 0))
            pt = sb.tile([128, 1024], BF16, tag="pT", bufs=pt_bufs,
                         name=f"pt{tau}{hp}{b}{half}")
            if diag:
                for j in range(2):
                    sl = slice(j * 512 + col0[j], (j + 1) * 512)
                    nc.scalar.activation(pt[:, sl], s[:, sl],
                                         mybir.ActivationFunctionType.Exp,
                                         scale=0.125)
                    # causal zeroing: only the 128x128 diagonal square has zeros
                    sq = slice(j * 512 + col0[j], j * 512 + col0[j] + 128)
                    nc.vector.tensor_mul(pt[:, sq], pt[:, sq], mask_sq[:])
            else:
                nc.scalar.activation(pt[:], s[:],
                                     mybir.ActivationFunctionType.Exp,
                                     scale=0.125)
            return (tau, hp, b, half, col0, pt)

        def emit_PV(item):
            tau, hp, b, half, col0, pt = item
            diag = b >= 2 * tau
            kmax = 4 * tau + 3
            otp = otp_tiles[(tau, hp)]
            for jj in range(2):
                u = 2 * b + jj
                h = hp * 2 + half
                c0 = col0[jj] if diag else 0
                nc.tensor.matmul(
                    otp[half][:, c0:512],
                    vw_aug[u][:, h * 65:h * 65 + 65],
                    pt[:, jj * 512 + c0:(jj + 1) * 512],
                    start=(u == 0), stop=(u == kmax),
                    skip_group_check=True)

        def emit_post_copy(tau, hp):
            """Drain otp psum to SBUF (frees the otp buffers for the next
            head-pair's PV accumulation)."""
            otp = otp_tiles[(tau, hp)]
            ot_sb = []
            for half in range(2):
                o1 = sb.tile([65, 512], POST, tag="otsb", bufs=6,
                             name=f"osb{tau}{hp}{half}")
                cpeng.tensor_copy(o1[:], otp[half][:])
                ot_sb.append(o1)
            return ot_sb

        def emit_post(tau, hp, ot_sb):
            """Normalize + transpose + stage one head-pair's output."""
            oo = oo_tiles[tau]
            for qs in range(4):
                if bf16_post:
                    otr = ps.tile([128, 260], BF16, tag="otr", bufs=2,
                                  name=f"otr{tau}{hp}{qs}")[:, 0:130]
                elif unified_psum:
                    otr = ps.tile([128, 1024], F32, tag="sps", bufs=sps_bufs,
                                  name=f"otr{tau}{hp}{qs}")[:, 0:130]
                else:
                    otr = ps.tile([128, 512], F32, tag="pj", bufs=2,
                                  name=f"otr{tau}{hp}{qs}")[:, 0:130]
                for half in range(2):
                    nc.tensor.transpose(
                        otr[:, 65 * half:65 * half + 65],
                        ot_sb[half][:, qs * 128:(qs + 1) * 128],
                        ident[0:65, 0:65])
                rc = sb.tile([128, 2], F32, tag="rc", bufs=4,
                             name=f"rc{tau}{hp}{qs}")
                nc.vector.reciprocal(rc[:], otr[:, 64:130:65])
                for half in range(2):
                    h = hp * 2 + half
                    nc.vector.tensor_scalar_mul(
                        oo[:, qs * COLS + h * 64:qs * COLS + (h + 1) * 64],
                        otr[:, 65 * half:65 * half + 64],
                        rc[:, half:half + 1])
                if hp == NHP - 1:
                    # store as soon as this 128-row block's last columns land
                    row = tau * 512 + qs * 128
                    nc.sync.dma_start(out[row:row + 128, :],
                                      oo[:, qs * COLS:(qs + 1) * COLS])

        def attn_tau(tau, scheds=(), proj_thunks=(), post_delay=2):
            """Software-pipelined attention stream for one q-tile.

            Emits S(i+lead) before PV(i) so the PE keeps streaming while
            ScalarE/DVE produce pt.  `scheds` is a list of [gen, units, end]:
            projection-unit generators dripped into the stream as PE filler
            (attention is ScalarE-bound, so the PE has spare cycles each
            element), with all `units` emitted by element index `end`; any
            remainder is flushed at the end.  proj_thunks (block mode) are
            interleaved at head-pair boundaries instead.  Head-pair posts are
            delayed `post_delay` elements so the PE transposes never wait on
            the otp->SBUF copy."""
            oo_tiles[tau] = sb.tile([128, 4 * COLS], F32, tag="oo", bufs=2,
                                    name=f"oo{tau}")
            nb = 2 * (tau + 1)
            seq = [(hp, b, half) for hp in range(NHP)
                   for b in range(nb) for half in range(2)]
            E = len(seq)
            fifo = []
            pv_done = 0
            next_copy = 0
            next_post = 0
            copy_at = [(hp + 1) * nb * 2 - 1 for hp in range(NHP)]
            post_at = [min(c + post_delay, E - 1) for c in copy_at]
            ot_sbs = {}
            proj_points = {(hp + 1) * nb * 2 - 1: i
                           for i, hp in enumerate(range(len(proj_thunks)))}
            scheds = [[g, u, max(1, min(e, E))] for g, u, e in scheds]
            done = [0] * len(scheds)

            U = sum(u for _, u, _ in scheds)

            def drip_one(si):
                g, u, _ = scheds[si]
                try:
                    next(g)
                    done[si] += 1
                    return True
                except StopIteration:
                    done[si] = u
                    return False

            def drip(i):
                # deadline minima first (in order), then pour the uniform
                # global budget into the earliest unfinished schedule so at
                # most two proj psum tiles are ever in flight
                for si, (g, u, e) in enumerate(scheds):
                    tgt = u if i + 1 >= e else ((i + 1) * u + e - 1) // e
                    while done[si] < tgt and drip_one(si):
                        pass
                gtgt = U if i + 1 >= E else ((i + 1) * U + E - 1) // E
                for si in range(len(scheds)):
                    while sum(done) < gtgt and done[si] < scheds[si][1]:
                        if not drip_one(si):
                            break

            def after_pv():
                nonlocal pv_done, next_copy, next_post
                while next_copy < NHP and pv_done >= copy_at[next_copy]:
                    ot_sbs[next_copy] = emit_post_copy(tau, next_copy)
                    next_copy += 1
                while next_post < NHP and pv_done >= post_at[next_post]:
                    emit_post(tau, next_post, ot_sbs.pop(next_post))
                    next_post += 1
                if pv_done in proj_points:
                    proj_thunks[proj_points[pv_done]]()
                pv_done += 1

            for i, (hp, b, half) in enumerate(seq):
                if (tau, hp) not in otp_tiles:
                    otp_tiles[(tau, hp)] = [
                        ps.tile([65, 512], F32, tag="ot", bufs=2,
                                name=f"ot{tau}{hp}{x}") for x in range(2)]
                fifo.append(emit_S(tau, hp, b, half))
                drip(i)
                while len(fifo) > lead:
                    emit_PV(fifo.pop(0))
                    after_pv()
            while fifo:
                emit_PV(fifo.pop(0))
                after_pv()
            drip(10 ** 9)  # flush any remaining proj units
            while next_copy < NHP:
                ot_sbs[next_copy] = emit_post_copy(tau, next_copy)
                next_copy += 1
            while next_post < NHP:
                emit_post(tau, next_post, ot_sbs.pop(next_post))
                next_post += 1

        if not do_proj and do_attn:
            # timing-only variant: initialize attention inputs so reads are legal
            for t in qwT + kwT:
                nc.gpsimd.memset(t[:], 0.0)
            for t in vw_aug:
                nc.gpsimd.memset(t[:], 1.0)
        for _rep in range(reps):
            w_loaded.clear()
            otp_tiles.clear()
            act_tiles.clear()
            if interleave == "drip" and do_proj and do_attn:
                proj_block("k", 0)
                proj_block("v", 0)
                proj_block("q", 0)
                for tau in range(NTAU):
                    E = NHP * 2 * (tau + 1) * 2
                    if tau < NTAU - 2 or (not spill and tau < NTAU - 1):
                        # host all of the next block's projections; one
                        # chained schedule so only one proj psum tile pipeline
                        # is in flight at a time
                        scheds = [[chain(proj_gen("q", tau + 1),
                                         proj_gen("k", tau + 1),
                                         proj_gen("v", tau + 1)), 96, E]]
                    elif spill and tau == NTAU - 2:
                        # spill part of k3/v3 into attn(3), which otherwise
                        # has no PE filler for its ScalarE-bound stream
                        scheds = [[chain(proj_gen("q", 3),
                                         proj_gen("k", 3, groups=[0]),
                                         proj_gen("v", 3, groups=[0, 1])), 56, E]]
                    elif spill and tau == NTAU - 1:
                        # spilled pieces, each before its first use
                        scheds = [[proj_gen("v", 3, groups=[2, 3]), 16, 14],
                                  [proj_gen("k", 3, groups=[1]), 8, 28],
                                  [proj_gen("k", 3, groups=[2]), 8, 44],
                                  [proj_gen("k", 3, groups=[3]), 8, 60]]
                    else:
                        scheds = []
                    attn_tau(tau, scheds=scheds, post_delay=post_delay)
            elif interleave == "fine" and do_proj and do_attn:
                proj_block("k", 0)
                proj_block("v", 0)
                proj_block("q", 0)
                for tau in range(NTAU):
                    if tau < NTAU - 1:
                        thunks = [lambda t=t, tau=tau: proj_block(t, tau + 1)
                                  for t in ("k", "v", "q")]
                    else:
                        thunks = []
                    attn_tau(tau, proj_thunks=thunks)
            elif interleave:
                for tau in range(NTAU):
                    if do_proj:
                        proj_block("k", tau)
                        proj_block("v", tau)
                        proj_block("q", tau)
                    if do_attn:
                        attn_tau(tau)
            else:
                if do_proj:
                    for tname in ("k", "v", "q"):
                        for lb in range(4):
                            proj_block(tname, lb)
                if do_attn:
                    for tau in range(NTAU):
                        attn_tau(tau)

    nc.compile()
    return nc


_NC_CACHE = None


def _get_nc():
    global _NC_CACHE
    if _NC_CACHE is None:
        _NC_CACHE = _build_kernel()
    return _NC_CACHE


def _tile_act(x):
    """[2048, 1024] fp32 -> [512, 4096] bf16 with [lb*128+p, d*512+c] layout."""
    t = x.reshape(4, 512, 8, 128).transpose(0, 3, 2, 1)  # [lb, p, d, c]
    return np.ascontiguousarray(t.reshape(512, 4096).astype(BF16_NP))


def _tile_w(w):
    """[1024, 512] fp32 -> [128, 4096] bf16 with [p, d*512+c] layout."""
    t = w.reshape(8, 128, 512).transpose(1, 0, 2)  # [p, d, c]
    return np.ascontiguousarray(t.reshape(128, 4096).astype(BF16_NP))


def make_in_maps(q, k, v, v_mask, q_mask, Wq, Wk, Wv):
    q = np.asarray(q, np.float32)
    k = np.asarray(k, np.float32)
    v = np.asarray(v, np.float32)
    v_mask = np.asarray(v_mask, np.float32)
    Wq = np.asarray(Wq, np.float32)
    Wk = np.asarray(Wk, np.float32)
    Wv = np.asarray(Wv, np.float32)
    in_maps = []
    for core in range(8):
        b, g = core // 2, core % 2
        cs = slice(g * COLS, (g + 1) * COLS)
        vp = v[b] * v_mask[b][:, None]
        in_maps.append({
            "qTt": _tile_act(q[b]),
            "kTt": _tile_act(k[b]),
            "vTt": _tile_act(vp),
            "wq": _tile_w(Wq[:, cs]),
            "wk": _tile_w(Wk[:, cs]),
            "wv": _tile_w(Wv[:, cs]),
            "vmask_t": np.ascontiguousarray(v_mask[b].reshape(NKSUB, 128).T),
        })
    return in_maps


def kernel(q, k, v, v_mask, q_mask, Wq, Wk, Wv):
    nc = _get_nc()
    in_maps = make_in_maps(q, k, v, v_mask, q_mask, Wq, Wk, Wv)
    res = run_bass_kernel_spmd(nc, in_maps, core_ids=list(range(8)))
    q_mask = np.asarray(q_mask, np.float32)
    out = np.empty((4, L, 2 * COLS), np.float32)
    for core in range(8):
        b, g = core // 2, core % 2
        out[b, :, g * COLS:(g + 1) * COLS] = res.results[core]["out"]
    out *= q_mask[:, :, None]
    return out


# revision 29
# speedup vs baseline: 1.1048x; 1.1048x over previous
"""Multi-head causal attention (B=4, L=2048, D=1024, H=16, dh=64) on 8 TRN2 NeuronCores.

Sharding: core i handles batch b = i//2 and head-group g = i%2 (8 heads each).
No cross-core communication needed: each core computes o[b, :, g*512:(g+1)*512].

Per-core dataflow (all layouts chosen so matmul contraction is on partitions):
  inputs (host-prepared, bf16, tiled so every DMA is a single contiguous ~1MB read):
    qTt/kTt/vTt [512, 4096]: row lb*128+p, col d*512+c  holds  x[b][lb*512+c, d*128+p]
    wq/wk/wv    [128, 4096]: row p,       col d*512+c  holds  W[d*128+p, c]
  projections (bf16 matmuls, fp32 psum):
    qwT/kwT [128(2 heads x 64dh), L] bf16;  vw_aug [128(Lk sub), 8*65] bf16 with a
    v_mask column appended per head (gives sum-of-exp for free in the PV matmul).
  attention, per q-tile tau of 512: a single software-pipelined stream over
  (hp, b, half) so the PE never waits on the exp chain:
    S^T[k,q] = kwT.T @ qwT per 128-k block (two K=64 heads row-packed in the PE),
    P^T = exp(S^T/8) via ScalarE (psum->sbuf, bf16), causal zeroing only of the
    128x128 diagonal squares via a DVE multiply with one precomputed triangular
    mask tile, then oT[65, 512] += vw_aug.T @ P^T accumulated over k blocks
    (row 64 = sum of exp).  The PV for stream element i is emitted after the S
    for element i+lead, so exp latency is hidden behind the next S matmuls.
    oT is transposed back via PE transpose; rows are scaled by 1/sumexp on DVE.
v_mask is pre-applied to v on host (and to the ones column via vmask_t on device);
q_mask is applied to the returned output on host.  Masks are {0,1} so this is exact.
"""
import numpy as np
import ml_dtypes
from contextlib import ExitStack
from itertools import chain

import concourse.bass as bass
import concourse.tile as tile
from concourse import bacc, mybir
from concourse.bass_utils import run_bass_kernel_spmd
from concourse.masks import make_identity

F32 = mybir.dt.float32
BF16 = mybir.dt.bfloat16
BF16_NP = ml_dtypes.bfloat16

L = 2048          # sequence length
D = 1024          # d_model
COLS = 512        # projection columns per core (8 heads x 64)
NKSUB = L // 128  # 16 k-subtiles
NTAU = L // 512   # 4 q-tiles
NHP = 4           # head pairs per core


def _build_kernel(interleave="drip", sps_bufs=2, pt_bufs=10, reps=1,
                  do_proj=True, do_attn=True, do_mm=True,
                  pj_share=False, unified_psum=False, lead=2, bf16_post=False,
                  pool_copies=False, act_q="sync", spill=True, post_delay=0,
                  mask_eng="vector", first_act_q="sync", mask_mode="mul"):
    nc = bacc.Bacc("TRN2", target_bir_lowering=False, debug=False, num_devices=8)

    qTt = nc.dram_tensor("qTt", [512, 4096], BF16, kind="ExternalInput").ap()
    kTt = nc.dram_tensor("kTt", [512, 4096], BF16, kind="ExternalInput").ap()
    vTt = nc.dram_tensor("vTt", [512, 4096], BF16, kind="ExternalInput").ap()
    wq = nc.dram_tensor("wq", [128, 4096], BF16, kind="ExternalInput").ap()
    wk = nc.dram_tensor("wk", [128, 4096], BF16, kind="ExternalInput").ap()
    wv = nc.dram_tensor("wv", [128, 4096], BF16, kind="ExternalInput").ap()
    vmask_t = nc.dram_tensor("vmask_t", [128, NKSUB], F32, kind="ExternalInput").ap()
    out = nc.dram_tensor("out", [L, COLS], F32, kind="ExternalOutput").ap()

    POST = BF16 if bf16_post else F32

    with tile.TileContext(nc) as tc, ExitStack() as ctx:
        sb = ctx.enter_context(tc.tile_pool(name="sb", bufs=1))
        ps = ctx.enter_context(tc.tile_pool(name="ps", bufs=1, space="PSUM"))

        # ---- persistent SBUF tensors ----
        w_t = {t: sb.tile([128, 4096], BF16, tag="w", bufs=3, name=f"w{t}")
               for t in ("q", "k", "v")}
        w_loaded = set()

        def load_weights(tname, split=False):
            if tname in w_loaded:
                return
            w_loaded.add(tname)
            src = {"q": wq, "k": wk, "v": wv}[tname]
            if split:
                # fine-grained first chunk so the d=0 matmuls start early
                nc.sync.dma_start(w_t[tname][:, 0:512], src[:, 0:512])
                nc.sync.dma_start(w_t[tname][:, 512:2048], src[:, 512:2048])
                nc.sync.dma_start(w_t[tname][:, 2048:4096], src[:, 2048:4096])
            else:
                nc.sync.dma_start(w_t[tname][:, 0:2048], src[:, 0:2048])
                nc.sync.dma_start(w_t[tname][:, 2048:4096], src[:, 2048:4096])

        vmask_sb = sb.tile([128, NKSUB], F32, tag="vm")
        nc.sync.dma_start(vmask_sb[:], vmask_t[:])
        ident = sb.tile([128, 128], F32, tag="id")
        make_identity(nc, ident[:])

        # one triangular mask for the 128x128 diagonal squares:
        # mask_sq[p, q] = 1 if q >= p else 0  (keep k <= q)
        mask_sq = sb.tile([128, 128], BF16, tag="mask")
        nc.gpsimd.memset(mask_sq[:], 1.0)
        nc.gpsimd.affine_select(
            out=mask_sq[:], in_=mask_sq[:], compare_op=mybir.AluOpType.is_ge,
            fill=0.0, base=0, channel_multiplier=-1, pattern=[[1, 128]])

        # bias-mask pattern: 0 where q >= p (keep), -3e4 where q < p, so the
        # diagonal-square S matmul can accumulate onto it (start=False) and
        # exp emits exact zeros without a separate mask multiply
        neg_tri = sb.tile([128, 128], F32, tag="ntri")
        nc.gpsimd.memset(neg_tri[:], 0.0)
        nc.gpsimd.affine_select(
            out=neg_tri[:], in_=neg_tri[:], compare_op=mybir.AluOpType.is_ge,
            fill=-30000.0, base=0, channel_multiplier=-1, pattern=[[1, 128]])

        qwT = [sb.tile([128, L], BF16, tag="qwT", bufs=NHP, name=f"qwT{hp}") for hp in range(NHP)]
        kwT = [sb.tile([128, L], BF16, tag="kwT", bufs=NHP, name=f"kwT{hp}") for hp in range(NHP)]
        vw_aug = [sb.tile([128, 8 * 65], BF16, tag="vwa", bufs=NKSUB, name=f"vwa{u}")
                  for u in range(NKSUB)]

        cpeng = nc.gpsimd if pool_copies else nc.vector
        act_tiles = {}

        def proj_setup(tname, lb):
            """Emit the input DMAs for one L-block (idempotent per block)."""
            key = (tname, lb)
            if key in act_tiles:
                return act_tiles[key]
            first = tname == "k" and lb == 0
            load_weights(tname, split=first)
            src = {"q": qTt, "k": kTt, "v": vTt}[tname]
            act = sb.tile([128, 4096], BF16, tag="act", bufs=6, name=f"a{tname}{lb}")
            # activations go on the Activation HWDGE queue so they don't
            # serialize behind weights/stores on the SP queue
            if lb == 0 and first_act_q == "scalar":
                dma_eng = nc.scalar
            else:
                dma_eng = nc.scalar if act_q == "scalar" else nc.sync
            if first:
                # split the very first load so d=0 matmuls start after 128KB
                dma_eng.dma_start(act[:, 0:512],
                                    src[lb * 128:(lb + 1) * 128, 0:512])
                dma_eng.dma_start(act[:, 512:2048],
                                    src[lb * 128:(lb + 1) * 128, 512:2048])
                dma_eng.dma_start(act[:, 2048:4096],
                                    src[lb * 128:(lb + 1) * 128, 2048:4096])
            else:
                dma_eng.dma_start(act[:], src[lb * 128:(lb + 1) * 128, :])
            act_tiles[key] = act
            return act

        def proj_gen(tname, lb, groups=None):
            """Generator of single-matmul projection units for (part of) one
            L-block.  groups selects head-pairs (q/k) or l-subtiles (v) so a
            block can be split across attention streams with per-piece
            deadlines.  Each next() emits one 512-col matmul (plus the
            psum->SBUF copy after a group's last matmul)."""
            act = proj_setup(tname, lb)
            if not do_mm:
                return iter(())
            if groups is None:
                groups = range(NHP)

            def units():
                wt = w_t[tname]
                pj_tag, pj_shape = (("sps", [128, 1024]) if pj_share
                                    else ("pj", [128, 512]))
                if tname != "v":
                    dst = qwT if tname == "q" else kwT
                    for hp in groups:
                        p = ps.tile(pj_shape, F32, tag=pj_tag,
                                    bufs=sps_bufs if pj_share else 2,
                                    name=f"pj{tname}{lb}{hp}")
                        p = p[:, 0:512]
                        for d in range(8):
                            nc.tensor.matmul(p[:],
                                             wt[:, d * 512 + hp * 128:d * 512 + (hp + 1) * 128],
                                             act[:, d * 512:(d + 1) * 512],
                                             start=(d == 0), stop=(d == 7),
                                             skip_group_check=True)
                            if d == 7:
                                cpeng.tensor_copy(
                                    dst[hp][:, lb * 512:(lb + 1) * 512], p[:])
                            yield
                else:
                    for ls in groups:
                        u = lb * 4 + ls
                        p = ps.tile(pj_shape, F32, tag=pj_tag,
                                    bufs=sps_bufs if pj_share else 2,
                                    name=f"pjv{u}")
                        p = p[:, 0:512]
                        for d in range(8):
                            nc.tensor.matmul(p[:],
                                             act[:, d * 512 + ls * 128:d * 512 + ls * 128 + 128],
                                             wt[:, d * 512:(d + 1) * 512],
                                             start=(d == 0), stop=(d == 7),
                                             skip_group_check=True)
                            if d == 7:
                                v3d = vw_aug[u][:].rearrange("p (h c) -> p h c", h=8)
                                cpeng.tensor_copy(
                                    v3d[:, :, 0:64],
                                    p[:].rearrange("p (h c) -> p h c", h=8))
                                cpeng.tensor_copy(
                                    v3d[:, :, 64:65].squeeze(2),
                                    vmask_sb[:, u:u + 1].broadcast_to([128, 8]))
                            yield
            return units()

        def proj_block(tname, lb):
            """Eagerly-drained projection block (preamble / non-drip modes)."""
            for _ in proj_gen(tname, lb):
                pass

        oo_tiles = {}
        otp_tiles = {}

        def emit_S(tau, hp, b, half):
            """S^T matmuls + exp (+ diag-square mask) for one (b, half).
            Returns the pt tile + metadata needed by emit_PV."""
            diag = b >= 2 * tau
            col0 = [128 * max(0, 2 * b + j - 4 * tau) for j in range(2)]
            s = ps.tile([128, 1024], F32, tag="sps", bufs=sps_bufs,
                        name=f"ss{tau}{hp}{b}{half}")
            bias = diag and mask_mode == "bias"
            if bias:
                # stage the causal bias into each diagonal square first; the
                # rest of the block is emitted start=True so the pattern only
                # needs the 128 square columns
                for j in range(2):
                    sq = slice(j * 512 + col0[j], j * 512 + col0[j] + 128)
                    nc.gpsimd.tensor_copy(s[:, sq], neg_tri[:])
            for j in range(2):
                u = 2 * b + j
                c0, c1 = j * 512 + col0[j], (j + 1) * 512
                qw = qwT[hp][64 * half:64 * half + 64,
                             tau * 512 + col0[j]:(tau + 1) * 512]
                kw = kwT[hp][64 * half:64 * half + 64, u * 128:(u + 1) * 128]
                if bias:
                    if c0 + 128 < c1:
                        nc.tensor.matmul(
                            s[:, c0 + 128:c1], kw, qw[:, 128:],
                            start=True, stop=True, skip_group_check=True,
                            tile_position=(64 * half, 0))
                    nc.tensor.matmul(
                        s[:, c0:c0 + 128], kw, qw[:, 0:128],
                        start=False, stop=True, skip_group_check=True,
                        tile_position=(64 * half, 0))
                else:
                    nc.tensor.matmul(
                        s[:, c0:c1], kw, qw,
                        start=True, stop=True, skip_group_check=True,
                        tile_position=(64 * half, 0))
            pt = sb.tile([128, 1024], BF16, tag="pT", bufs=pt_bufs,
                         name=f"pt{tau}{hp}{b}{half}")
            if diag:
                for j in range(2):
                    sl = slice(j * 512 + col0[j], (j + 1) * 512)
                    nc.scalar.activation(pt[:, sl], s[:, sl],
                                         mybir.ActivationFunctionType.Exp,
                                         scale=0.125)
                    if mask_mode != "bias":
                        sq = slice(j * 512 + col0[j], j * 512 + col0[j] + 128)
                        getattr(nc, mask_eng).tensor_mul(pt[:, sq], pt[:, sq],
                                                         mask_sq[:])
            else:
                nc.scalar.activation(pt[:], s[:],
                                     mybir.ActivationFunctionType.Exp,
                                     scale=0.125)
            return (tau, hp, b, half, col0, pt)

        def emit_PV(item):
            tau, hp, b, half, col0, pt = item
            diag = b >= 2 * tau
            kmax = 4 * tau + 3
            otp = otp_tiles[(tau, hp)]
            for jj in range(2):
                u = 2 * b + jj
                h = hp * 2 + half
                c0 = col0[jj] if diag else 0
                nc.tensor.matmul(
                    otp[half][:, c0:512],
                    vw_aug[u][:, h * 65:h * 65 + 65],
                    pt[:, jj * 512 + c0:(jj + 1) * 512],
                    start=(u == 0), stop=(u == kmax),
                    skip_group_check=True)

        def emit_post_copy(tau, hp):
            """Drain otp psum to SBUF (frees the otp buffers for the next
            head-pair's PV accumulation)."""
            otp = otp_tiles[(tau, hp)]
            ot_sb = []
            for half in range(2):
                o1 = sb.tile([65, 512], POST, tag="otsb", bufs=6,
                             name=f"osb{tau}{hp}{half}")
                cpeng.tensor_copy(o1[:], otp[half][:])
                ot_sb.append(o1)
            return ot_sb

        def emit_post(tau, hp, ot_sb):
            """Normalize + transpose + stage one head-pair's output."""
            oo = oo_tiles[tau]
            for qs in range(4):
                if bf16_post:
                    otr = ps.tile([128, 260], BF16, tag="otr", bufs=2,
                                  name=f"otr{tau}{hp}{qs}")[:, 0:130]
                elif unified_psum:
                    otr = ps.tile([128, 1024], F32, tag="sps", bufs=sps_bufs,
                                  name=f"otr{tau}{hp}{qs}")[:, 0:130]
                else:
                    otr = ps.tile([128, 512], F32, tag="pj", bufs=2,
                                  name=f"otr{tau}{hp}{qs}")[:, 0:130]
                for half in range(2):
                    nc.tensor.transpose(
                        otr[:, 65 * half:65 * half + 65],
                        ot_sb[half][:, qs * 128:(qs + 1) * 128],
                        ident[0:65, 0:65])
                rc = sb.tile([128, 2], F32, tag="rc", bufs=4,
                             name=f"rc{tau}{hp}{qs}")
                nc.vector.reciprocal(rc[:], otr[:, 64:130:65])
                for half in range(2):
                    h = hp * 2 + half
                    nc.vector.tensor_scalar_mul(
                        oo[:, qs * COLS + h * 64:qs * COLS + (h + 1) * 64],
                        otr[:, 65 * half:65 * half + 64],
                        rc[:, half:half + 1])
                if hp == NHP - 1:
                    # store as soon as this 128-row block's last columns land
                    row = tau * 512 + qs * 128
                    nc.sync.dma_start(out[row:row + 128, :],
                                      oo[:, qs * COLS:(qs + 1) * COLS])

        def attn_tau(tau, scheds=(), proj_thunks=(), post_delay=2):
            """Software-pipelined attention stream for one q-tile.

            Emits S(i+lead) before PV(i) so the PE keeps streaming while
            ScalarE/DVE produce pt.  `scheds` is a list of [gen, units, end]:
            projection-unit generators dripped into the stream as PE filler
            (attention is ScalarE-bound, so the PE has spare cycles each
            element), with all `units` emitted by element index `end`; any
            remainder is flushed at the end.  proj_thunks (block mode) are
            interleaved at head-pair boundaries instead.  Head-pair posts are
            delayed `post_delay` elements so the PE transposes never wait on
            the otp->SBUF copy."""
            oo_tiles[tau] = sb.tile([128, 4 * COLS], F32, tag="oo", bufs=2,
                                    name=f"oo{tau}")
            nb = 2 * (tau + 1)
            seq = [(hp, b, half) for hp in range(NHP)
                   for b in range(nb) for half in range(2)]
            E = len(seq)
            fifo = []
            pv_done = 0
            next_copy = 0
            next_post = 0
            copy_at = [(hp + 1) * nb * 2 - 1 for hp in range(NHP)]
            post_at = [min(c + post_delay, E - 1) for c in copy_at]
            ot_sbs = {}
            proj_points = {(hp + 1) * nb * 2 - 1: i
                           for i, hp in enumerate(range(len(proj_thunks)))}
            scheds = [[g, u, max(1, min(e, E))] for g, u, e in scheds]
            done = [0] * len(scheds)

            U = sum(u for _, u, _ in scheds)

            def drip_one(si):
                g, u, _ = scheds[si]
                try:
                    next(g)
                    done[si] += 1
                    return True
                except StopIteration:
                    done[si] = u
                    return False

            def drip(i):
                # deadline minima first (in order), then pour the uniform
                # global budget into the earliest unfinished schedule so at
                # most two proj psum tiles are ever in flight
                for si, (g, u, e) in enumerate(scheds):
                    tgt = u if i + 1 >= e else ((i + 1) * u + e - 1) // e
                    while done[si] < tgt and drip_one(si):
                        pass
                gtgt = U if i + 1 >= E else ((i + 1) * U + E - 1) // E
                for si in range(len(scheds)):
                    while sum(done) < gtgt and done[si] < scheds[si][1]:
                        if not drip_one(si):
                            break

            def after_pv():
                nonlocal pv_done, next_copy, next_post
                while next_copy < NHP and pv_done >= copy_at[next_copy]:
                    ot_sbs[next_copy] = emit_post_copy(tau, next_copy)
                    next_copy += 1
                while next_post < NHP and pv_done >= post_at[next_post]:
                    emit_post(tau, next_post, ot_sbs.pop(next_post))
                    next_post += 1
                if pv_done in proj_points:
                    proj_thunks[proj_points[pv_done]]()
                pv_done += 1

            for i, (hp, b, half) in enumerate(seq):
                if (tau, hp) not in otp_tiles:
                    otp_tiles[(tau, hp)] = [
                        ps.tile([65, 512], F32, tag="ot", bufs=2,
                                name=f"ot{tau}{hp}{x}") for x in range(2)]
                fifo.append(emit_S(tau, hp, b, half))
                drip(i)
                while len(fifo) > lead:
                    emit_PV(fifo.pop(0))
                    after_pv()
            while fifo:
                emit_PV(fifo.pop(0))
                after_pv()
            drip(10 ** 9)  # flush any remaining proj units
            while next_copy < NHP:
                ot_sbs[next_copy] = emit_post_copy(tau, next_copy)
                next_copy += 1
            while next_post < NHP:
                emit_post(tau, next_post, ot_sbs.pop(next_post))
                next_post += 1

        if not do_proj and do_attn:
            # timing-only variant: initialize attention inputs so reads are legal
            for t in qwT + kwT:
                nc.gpsimd.memset(t[:], 0.0)
            for t in vw_aug:
                nc.gpsimd.memset(t[:], 1.0)
        for _rep in range(reps):
            w_loaded.clear()
            otp_tiles.clear()
            act_tiles.clear()
            if interleave == "drip" and do_proj and do_attn:
                proj_block("k", 0)
                proj_block("v", 0)
                proj_block("q", 0)
                for tau in range(NTAU):
                    E = NHP * 2 * (tau + 1) * 2
                    if tau < NTAU - 2 or (not spill and tau < NTAU - 1):
                        # host all of the next block's projections; one
                        # chained schedule so only one proj psum tile pipeline
                        # is in flight at a time
                        scheds = [[chain(proj_gen("q", tau + 1),
                                         proj_gen("k", tau + 1),
                                         proj_gen("v", tau + 1)), 96, E]]
                    elif spill and tau == NTAU - 2:
                        # spill part of k3/v3 into attn(3), which otherwise
                        # has no PE filler for its ScalarE-bound stream
                        scheds = [[chain(proj_gen("q", 3),
                                         proj_gen("k", 3, groups=[0]),
                                         proj_gen("v", 3, groups=[0, 1])), 56, E]]
                    elif spill and tau == NTAU - 1:
                        # spilled pieces, each before its first use
                        scheds = [[proj_gen("v", 3, groups=[2, 3]), 16, 14],
                                  [proj_gen("k", 3, groups=[1]), 8, 28],
                                  [proj_gen("k", 3, groups=[2]), 8, 44],
                                  [proj_gen("k", 3, groups=[3]), 8, 60]]
                    else:
                        scheds = []
                    attn_tau(tau, scheds=scheds, post_delay=post_delay)
            elif interleave == "fine" and do_proj and do_attn:
                proj_block("k", 0)
                proj_block("v", 0)
                proj_block("q", 0)
                for tau in range(NTAU):
                    if tau < NTAU - 1:
                        thunks = [lambda t=t, tau=tau: proj_block(t, tau + 1)
                                  for t in ("k", "v", "q")]
                    else:
                        thunks = []
                    attn_tau(tau, proj_thunks=thunks)
            elif interleave:
                for tau in range(NTAU):
                    if do_proj:
                        proj_block("k", tau)
                        proj_block("v", tau)
                        proj_block("q", tau)
                    if do_attn:
                        attn_tau(tau)
            else:
                if do_proj:
                    for tname in ("k", "v", "q"):
                        for lb in range(4):
                            proj_block(tname, lb)
                if do_attn:
                    for tau in range(NTAU):
                        attn_tau(tau)

    nc.compile()
    return nc


_NC_CACHE = None


def _get_nc():
    global _NC_CACHE
    if _NC_CACHE is None:
        _NC_CACHE = _build_kernel()
    return _NC_CACHE


def _tile_act(x):
    """[2048, 1024] fp32 -> [512, 4096] bf16 with [lb*128+p, d*512+c] layout."""
    t = x.reshape(4, 512, 8, 128).transpose(0, 3, 2, 1)  # [lb, p, d, c]
    return np.ascontiguousarray(t.reshape(512, 4096).astype(BF16_NP))


def _tile_w(w):
    """[1024, 512] fp32 -> [128, 4096] bf16 with [p, d*512+c] layout."""
    t = w.reshape(8, 128, 512).transpose(1, 0, 2)  # [p, d, c]
    return np.ascontiguousarray(t.reshape(128, 4096).astype(BF16_NP))


def make_in_maps(q, k, v, v_mask, q_mask, Wq, Wk, Wv):
    q = np.asarray(q, np.float32)
    k = np.asarray(k, np.float32)
    v = np.asarray(v, np.float32)
    v_mask = np.asarray(v_mask, np.float32)
    Wq = np.asarray(Wq, np.float32)
    Wk = np.asarray(Wk, np.float32)
    Wv = np.asarray(Wv, np.float32)
    in_maps = []
    for core in range(8):
        b, g = core // 2, core % 2
        cs = slice(g * COLS, (g + 1) * COLS)
        vp = v[b] * v_mask[b][:, None]
        in_maps.append({
            "qTt": _tile_act(q[b]),
            "kTt": _tile_act(k[b]),
            "vTt": _tile_act(vp),
            "wq": _tile_w(Wq[:, cs]),
            "wk": _tile_w(Wk[:, cs]),
            "wv": _tile_w(Wv[:, cs]),
            "vmask_t": np.ascontiguousarray(v_mask[b].reshape(NKSUB, 128).T),
        })
    return in_maps


def kernel(q, k, v, v_mask, q_mask, Wq, Wk, Wv):
    nc = _get_nc()
    in_maps = make_in_maps(q, k, v, v_mask, q_mask, Wq, Wk, Wv)
    res = run_bass_kernel_spmd(nc, in_maps, core_ids=list(range(8)))
    q_mask = np.asarray(q_mask, np.float32)
    out = np.empty((4, L, 2 * COLS), np.float32)
    for core in range(8):
        b, g = core // 2, core % 2
        out[b, :, g * COLS:(g + 1) * COLS] = res.results[core]["out"]
    out *= q_mask[:, :, None]
    return out
